# revision 21
# baseline (speedup 1.0000x reference)
"""Trainium2 Bass kernel for nn_DiGCNLayerAtt (directed GCN layer with
adjacency-masked attention), batch-parallel over 8 NeuronCores.

Math (same scaling tricks as before, validated vs reference):
  softmax denominator / renormalization / row-max are uniform positive
  per-row scalings of att; LayerNorm is invariant to them. So the kernel
  computes attu[n,m] = exp(u_raw[n,m]/16) * A[n,m] and LayerNorms the
  unnormalized context  ctx = attu_left @ hl + diag(attu)*hs + attu_right @ hr.

v4 changes on top of the v2 structure below:
  * u-cells run as 3 fp8 DoubleRow matmuls with error feedback
    (h8@h8 + h8@r8 + r8@h8, r8 = fp8(h16 - fp8(h16))): 768 PE cycles
    per cell vs 1024 f16, logit error ~5e-3 (output absmax ~1.3e-3).
    fp8 WITHOUT feedback fails the 2e-2 gate: quantization noise does
    not average in attention-weighted sums (random-sign averaging
    shrinks signal and noise equally), so hl/hr/att/e stay f16.
  * LayerNorm rsqrt seeds from exp(-0.5*ln(v)) instead of Sqrt —
    Sqrt shares no ACT table with Exp, so the old path reloaded
    activation tables (~1.3us each) twice per chunk.
  * The self term dv*hs is fused into the ctx evacuation
    (scalar_tensor_tensor with accum_out), dropping 16 DVE ops.

v2 structure — exploits u's symmetry (u = h h^T):
  * u is computed only for upper-triangular 128-block cells (m-block <=
    n-block): 40 of 64 (m, chunk) cells. exp runs only on those cells
    (ACT work -37%).
  * The lower-triangle stationaries attu^T[m,n] = e[m,n]*A[n,m] (m>n) are
    rebuilt from the upper cells' e via batched DMA-xbar 128x128 block
    transposes (dma_start_transpose, 16 blocks per dispatch, 6 dispatches
    total) — no PE/DVE cost — then masked against A^T on DVE.
  * u's diagonal (|h_n|^2) is precomputed on the host (exact), so no
    on-device diag extraction; the diag block of e overflows to inf in
    f16, and strict-triangle copy_predicated masks (never touching the
    diagonal) keep inf/NaN out of every matmul operand.
  * mask-multiplies are batched 4 rows at a time ([128,2048] ops); a few
    late-consumed ones run on the otherwise-idle Pool engine.
  * normalize+relu is fused into one ACT op per n-block via per-partition
    scale/bias (istd, -mu*istd); output stored f16 (~5e-4 err vs 2e-2 tol).
  * A^T streams in [128, 4x512] slices (16 dispatches/rep); e-storage is
    split per chunk-column group and hdir double-buffered so consecutive
    repeats of the compute chain pipeline instead of serializing.

Per-core flow (batch b on core b), chunk = 512 n-cols, group g = 4
m-blocks:  hdir for all m; then per group g: u-cells (c-major), et
transposes, ctx chunk g (direct gg<=g from e_g, lower gg>g from et),
per-chunk evacuation+stats+relu+store (per-block chains on the last
chunk to shrink the kernel tail).
"""
import sys

sys.path.insert(0, "/opt/trn_rl_repo")

import numpy as np

import concourse.bass as bass
import concourse.tile as tile
from concourse import bacc, mybir
from concourse import bass_utils
from concourse.bass_interp import get_hw_module

F32 = mybir.dt.float32
F16 = mybir.dt.float16
F8 = mybir.dt.float8e4
U32 = mybir.dt.uint32
ALU = mybir.AluOpType
ACTF = mybir.ActivationFunctionType
DR = mybir.MatmulPerfMode.DoubleRow

B, N, H = 8, 2048, 256
NT = N // 128           # 16 m/n blocks of 128
CHUNK = 512
NG = 4                  # groups of 4 blocks; also number of chunks
GS = NT // NG           # 4 blocks per group
TEMPER_INV = 1.0 / float(np.sqrt(H))  # 1/16
EPS_LN = 1e-12

# mask-multiplies routed to the (slow but otherwise idle) GPSIMD engine:
# picked to be late-consumed within their chunk so Pool latency hides.
POOL_MASKS = {(0, 3), (0, 2), (1, 3), (1, 2), (2, 3)}


def build_program(apply_ln: bool, repeat: int = 1):
    nc = bacc.Bacc("TRN2", target_bir_lowering=False, debug=False, num_devices=B)

    hT_d = nc.dram_tensor("hT", [H, N], F16, kind="ExternalInput")
    h8_d = nc.dram_tensor("h8", [128, 2 * N], F8, kind="ExternalInput")
    r8_d = nc.dram_tensor("r8", [128, 2 * N], F8, kind="ExternalInput")
    AT_d = nc.dram_tensor("AT", [N, N], F16, kind="ExternalInput")
    wcat_d = nc.dram_tensor("wcat", [H, 3 * H], F16, kind="ExternalInput")
    bias_d = nc.dram_tensor("bias_cat", [128, 3 * H], F32, kind="ExternalInput")
    masklo_d = nc.dram_tensor("masklo", [128, 128], mybir.dt.uint8, kind="ExternalInput")
    maskup_d = nc.dram_tensor("maskup", [128, 128], mybir.dt.uint8, kind="ExternalInput")
    adiag_d = nc.dram_tensor("adiag", [128, NT], F32, kind="ExternalInput")
    udiag_d = nc.dram_tensor("udiag", [128, NT], F32, kind="ExternalInput")
    if apply_ln:
        lnw_d = nc.dram_tensor("lnw_bc", [128, H], F32, kind="ExternalInput")
        lnb_d = nc.dram_tensor("lnb_bc", [128, H], F32, kind="ExternalInput")
    out_d = nc.dram_tensor("out", [N, H], F16, kind="ExternalOutput")

    with tile.TileContext(nc) as tc:
        with (
            tc.tile_pool(name="consts", bufs=1) as cpool,
            tc.tile_pool(name="attup", bufs=6) as apool,
            tc.tile_pool(name="hlp", bufs=2) as hpool,
            tc.tile_pool(name="atp", bufs=7) as atpool,
            tc.tile_pool(name="etp", bufs=3) as etpool,
            tc.tile_pool(name="small", bufs=5) as smpool,
            tc.tile_pool(name="outp", bufs=3) as opool,
            tc.tile_pool(name="upsum", bufs=4, space=bass.MemorySpace.PSUM) as upool,
            tc.tile_pool(name="ctxpsum", bufs=2, space=bass.MemorySpace.PSUM) as xpool,
        ):
            v = nc.vector
            sc = nc.scalar
            g_ = nc.gpsimd

            # ---- constants / persistent SBUF ----
            hT0 = cpool.tile([128, N], F16, tag="hT0")
            hT1 = cpool.tile([128, N], F16, tag="hT1")
            nc.sync.dma_start(hT0[:, 0:1024], hT_d.ap()[0:128, 0:1024])
            nc.sync.dma_start(hT1[:, 0:1024], hT_d.ap()[128:256, 0:1024])
            nc.sync.dma_start(hT0[:, 1024:N], hT_d.ap()[0:128, 1024:N])
            nc.sync.dma_start(hT1[:, 1024:N], hT_d.ap()[128:256, 1024:N])
            h8c = cpool.tile([128, 2 * N], F8, tag="h8c")
            r8c = cpool.tile([128, 2 * N], F8, tag="r8c")
            nc.sync.dma_start(h8c[:], h8_d.ap())
            nc.sync.dma_start(r8c[:], r8_d.ap())
            h8k = h8c[:].rearrange("p (two n) -> p two n", two=2)
            r8k = r8c[:].rearrange("p (two n) -> p two n", two=2)
            wcat0 = cpool.tile([128, 3 * H], F16, tag="wcat0")
            wcat1 = cpool.tile([128, 3 * H], F16, tag="wcat1")
            nc.sync.dma_start(wcat0[:], wcat_d.ap()[0:128, :])
            nc.sync.dma_start(wcat1[:], wcat_d.ap()[128:256, :])
            bias_cat = cpool.tile([128, 3 * H], F32, tag="bias_cat")
            nc.sync.dma_start(bias_cat[:], bias_d.ap())
            masklo = cpool.tile([128, 128], mybir.dt.uint8, tag="masklo")
            maskup = cpool.tile([128, 128], mybir.dt.uint8, tag="maskup")
            nc.sync.dma_start(masklo[:], masklo_d.ap())
            nc.sync.dma_start(maskup[:], maskup_d.ap())
            adiag = cpool.tile([128, NT], F32, tag="adiag")
            udiag = cpool.tile([128, NT], F32, tag="udiag")
            nc.sync.dma_start(adiag[:], adiag_d.ap())
            nc.sync.dma_start(udiag[:], udiag_d.ap())
            if apply_ln:
                lnw = cpool.tile([128, H], F32, tag="lnw")
                lnb = cpool.tile([128, H], F32, tag="lnb")
                nc.sync.dma_start(lnw[:], lnw_d.ap())
                nc.sync.dma_start(lnb[:], lnb_d.ap())

            # e storage per chunk-column group: cells (alpha, c) at
            # e_g[c][:, alpha*512].  Per-group tiles let rep i+1's u-cells
            # overwrite group c as soon as rep i's chunk-c masks are done
            # (wavefront overlap across repeats).
            e_g = [cpool.tile([128, (4 * c + 4) * CHUNK], F16, tag=f"e_g{c}",
                              name=f"e_g{c}") for c in range(NG)]
            ctx_sb = cpool.tile([128, NT * H], F32, tag="ctx_sb")
            out_sb = cpool.tile([128, NT * H], F16, tag="out_sb")
            out_v = out_d.ap().rearrange("(t p) h -> p t h", p=128)
            out_sv = out_sb[:].rearrange("p (t h) -> p t h", h=H)
            sum_b = cpool.tile([128, NT], F32, tag="sum_b")
            mu_b = cpool.tile([128, NT], F32, tag="mu_b")
            var_b = cpool.tile([128, NT], F32, tag="var_b")
            qmagic = cpool.tile([128, GS], U32, tag="qmagic")
            g_.memset(qmagic[:], 0x5F3759DF)

            # DRAM view of A^T: [panel-group gg][row p][panel-in-group b][col n]
            atd = AT_d.ap().rearrange("(gg b p) n -> gg p b n", p=128, b=GS)

            def emit_ln(nts):
                # LayerNorm stats + fused normalize/relu for n-blocks `nts`
                # (contiguous column range of the per-block stat tensors).
                k = len(nts)
                csl = slice(nts[0], nts[0] + k)
                veps = smpool.tile([128, k], F32, tag="veps",
                                   name=f"veps{nts[0]}")
                v.tensor_scalar(veps[:], var_b[:, csl], 1.0 / H, EPS_LN,
                                op0=ALU.mult, op1=ALU.add)
                # rsqrt seed via exponent bit-trick on DVE (no ACT table):
                # istd0 = bitcast(0x5f3759df - (bits(veps) >> 1)), then two
                # Newton steps (seed err <=3.4% -> ~4e-6).
                dsh = smpool.tile([128, k], U32, tag="dsh",
                                  name=f"dsh{nts[0]}")
                v.tensor_scalar(dsh[:], veps[:].bitcast(U32), 1, None,
                                op0=ALU.logical_shift_right)
                ib = smpool.tile([128, k], U32, tag="ib",
                                 name=f"ib{nts[0]}")
                v.tensor_tensor(ib[:], qmagic[:, 0:k], dsh[:],
                                op=ALU.subtract)
                inv0 = ib[:].bitcast(F32)
                # one Newton step: seed err <=3.4% -> istd err ~0.17%, and a
                # half-step bias correction folded into the 1.5 constant
                # keeps the per-row scale error ~1e-3 (vs 2e-2 gate).
                nw1 = smpool.tile([128, k], F32, tag="nw1",
                                  name=f"nw1{nts[0]}")
                istd = smpool.tile([128, k], F32, tag="istd",
                                   name=f"istd{nts[0]}")
                v.tensor_tensor(nw1[:], inv0, inv0, op=ALU.mult)
                v.tensor_tensor(nw1[:], veps[:], nw1[:], op=ALU.mult)
                v.tensor_scalar(nw1[:], nw1[:], -0.5, 1.5, op0=ALU.mult,
                                op1=ALU.add)
                v.tensor_tensor(istd[:], inv0, nw1[:], op=ALU.mult)
                nmu = smpool.tile([128, k], F32, tag="nmu",
                                  name=f"nmu{nts[0]}")
                v.scalar_tensor_tensor(nmu[:], mu_b[:, csl], -1.0, istd[:],
                                       op0=ALU.mult, op1=ALU.mult)
                for j, nt_i in enumerate(nts):
                    if apply_ln:
                        pre = opool.tile([128, H], F32, tag="pre")
                        v.tensor_scalar(pre[:],
                                        ctx_sb[:, nt_i * H:(nt_i + 1) * H],
                                        mu_b[:, nt_i:nt_i + 1],
                                        istd[:, j:j + 1],
                                        op0=ALU.subtract, op1=ALU.mult)
                        v.tensor_tensor(pre[:], pre[:], lnw[:], op=ALU.mult)
                        v.tensor_tensor(pre[:], pre[:], lnb[:], op=ALU.add)
                        sc.activation(out_sb[:, nt_i * H:(nt_i + 1) * H],
                                      pre[:], ACTF.Relu)
                    else:
                        sc.activation(out_sb[:, nt_i * H:(nt_i + 1) * H],
                                      ctx_sb[:, nt_i * H:(nt_i + 1) * H],
                                      ACTF.Relu, bias=nmu[:, j:j + 1],
                                      scale=istd[:, j:j + 1])

            def emit_all():
                at_tiles = {}
                premask = {}
                hlsr_all = hpool.tile([128, NT * 3 * H], F16, tag="hlsr",
                                      name="hlsr_all")
                hl = [hlsr_all[:, (3 * m) * H:(3 * m + 1) * H]
                      for m in range(NT)]
                hs = [hlsr_all[:, (3 * m + 1) * H:(3 * m + 2) * H]
                      for m in range(NT)]
                hr = [hlsr_all[:, (3 * m + 2) * H:(3 * m + 3) * H]
                      for m in range(NT)]

                def load_at(c, gg):
                    # [128, 4 panels x 512 cols] slice of A^T for (chunk c, group gg)
                    t = atpool.tile([128, GS * CHUNK], F16, tag="at",
                                    name=f"at{c}_{gg}")
                    nc.sync.dma_start(
                        t[:].rearrange("p (b q) -> p b q", q=CHUNK),
                        atd[gg, :, :, c * CHUNK:(c + 1) * CHUNK])
                    at_tiles[c, gg] = t

                for gg in range(NG):
                    load_at(0, gg)

                # ---- phase A: hdir for all m ----
                for m in range(NT):
                    ms = bass.ts(m, 128)
                    hp = xpool.tile([128, 2 * CHUNK], F32, tag="ctx",
                                    name=f"hp{m}")
                    nc.tensor.matmul(hp[:, 0:512], hT0[:, ms], wcat0[:, 0:512],
                                     start=True, stop=False)
                    nc.tensor.matmul(hp[:, 512:768], hT0[:, ms],
                                     wcat0[:, 512:768], start=True, stop=False)
                    nc.tensor.matmul(hp[:, 0:512], hT1[:, ms], wcat1[:, 0:512],
                                     start=False, stop=True)
                    nc.tensor.matmul(hp[:, 512:768], hT1[:, ms],
                                     wcat1[:, 512:768], start=False, stop=True)
                    v.tensor_tensor(hlsr_all[:, 3 * m * H:(3 * m + 3) * H],
                                    hp[:, 0:768], bias_cat[:], op=ALU.add)

                # de/dv for self term (tiny, do once)
                de = smpool.tile([128, NT], F32, tag="de")
                sc.activation(de[:], udiag[:], ACTF.Exp, scale=TEMPER_INV)
                dv = smpool.tile([128, NT], F32, tag="dv")
                v.tensor_tensor(dv[:], de[:], adiag[:], op=ALU.mult)

                et_tiles = {}
                for g in range(NG):
                    # ---- u-cells for group g (c-major) + et transposes ----
                    for c in range(g, NG):
                        for al in range(GS):
                            a = g * GS + al
                            as_ = bass.ts(a, 128)
                            u = upool.tile([128, CHUNK], F32, tag="u")
                            csl = slice(c * CHUNK, (c + 1) * CHUNK)
                            asl = slice(a * 128, (a + 1) * 128)
                            nc.tensor.matmul(u[:], h8k[:, :, asl],
                                             h8k[:, :, csl],
                                             start=True, stop=False,
                                             perf_mode=DR)
                            nc.tensor.matmul(u[:], h8k[:, :, asl],
                                             r8k[:, :, csl],
                                             start=False, stop=False,
                                             perf_mode=DR)
                            nc.tensor.matmul(u[:], r8k[:, :, asl],
                                             h8k[:, :, csl],
                                             start=False, stop=True,
                                             perf_mode=DR)
                            off = a * CHUNK
                            sc.activation(e_g[c][:, off:off + CHUNK], u[:],
                                          ACTF.Exp, scale=TEMPER_INV)
                        if c > g:
                            # blocks E_{alpha in g, beta in group c} transposed
                            et = etpool.tile([128, 4 * CHUNK], F16, tag="et",
                                             name=f"et{g}_{c}")
                            sl = g * GS * CHUNK
                            nc.sync.dma_start_transpose(
                                et[:].rearrange("p (j q) -> p j q", q=128),
                                e_g[c][:, sl:sl + 4 * CHUNK])
                            et_tiles[g, c] = et

                    # ---- ctx chunk g ----
                    if g + 1 < NG:
                        for gg in range(NG):
                            load_at(g + 1, gg)
                    ctx = xpool.tile([128, 4 * H], F32, tag="ctx",
                                     name=f"ctx{g}")
                    attus = []
                    for gg in range(NG):
                        if (g, gg) in premask:
                            attus.append(premask.pop((g, gg)))
                            continue
                        attu = apool.tile([128, 4 * CHUNK], F16, tag="attu",
                                          name=f"attu{g}_{gg}")
                        if gg <= g:
                            # direct: cells (beta in gg, chunk g) from e_all
                            sl = gg * GS * CHUNK
                            ev = e_g[g][:, sl:sl + 4 * CHUNK].rearrange(
                                "p (b c) -> p b c", c=CHUNK)
                            av = at_tiles[g, gg][:].rearrange(
                                "p (b c) -> p b c", c=CHUNK)
                            ov = attu[:].rearrange("p (b c) -> p b c", c=CHUNK)
                            meng = g_ if (g, gg) in POOL_MASKS else v
                            meng.tensor_tensor(ov, ev, av, op=ALU.mult)
                        else:
                            # lower: transposed blocks * A^T
                            et = et_tiles[g, gg]
                            ev = et[:].rearrange("p (a b q) -> p a b q",
                                                 b=GS, q=128)
                            av = at_tiles[g, gg][:].rearrange(
                                "p (b a q) -> p a b q", a=GS, q=128)
                            ov = attu[:].rearrange("p (a b q) -> p a b q",
                                                   b=GS, q=128)
                            meng = g_ if (g, gg) in POOL_MASKS else v
                            meng.tensor_tensor(ov, ev, av, op=ALU.mult)
                        attus.append(attu)

                    def st(ns, beta):
                        gg, bl = beta // GS, beta % GS
                        attu = attus[gg]
                        if gg <= g:
                            o = bl * CHUNK + ns * 128
                        else:
                            o = (ns * GS + bl) * 128
                        return attu[:, o:o + 128]

                    for beta in range(NT):
                        # beta's own diagonal block is n-block == beta,
                        # i.e. ns == beta - g*GS, only when beta in chunk g.
                        dns = beta - g * GS
                        for ns in range(GS):
                            nt_i = g * GS + ns
                            first = (beta == 0) and (ns % 2 == 0)
                            last = (beta == NT - 1) and (ns % 2 == 1)
                            o = ctx[:, ns * H:(ns + 1) * H]
                            if ns == dns:
                                azt = smpool.tile([128, 256], F16, tag="az",
                                                  name=f"az{g}_{beta}")
                                g_.memset(azt[:], 0)
                                alo = azt[:, 0:128]
                                aup = azt[:, 128:256]
                                v.copy_predicated(alo, masklo[:], st(ns, beta))
                                v.copy_predicated(aup, maskup[:], st(ns, beta))
                                nc.tensor.matmul(o, alo, hl[beta][:],
                                                 start=first, stop=False)
                                nc.tensor.matmul(o, aup, hr[beta][:],
                                                 start=False, stop=last)
                            elif beta > nt_i:
                                nc.tensor.matmul(o, st(ns, beta), hl[beta][:],
                                                 start=first, stop=last)
                            else:
                                nc.tensor.matmul(o, st(ns, beta), hr[beta][:],
                                                 start=first, stop=last)

                    # ---- phase C: evacuate + self term + stats ----
                    for ns in range(GS):
                        nt_i = g * GS + ns
                        cs = ctx_sb[:, nt_i * H:(nt_i + 1) * H]
                        v.scalar_tensor_tensor(
                            cs, hs[nt_i][:], dv[:, nt_i:nt_i + 1],
                            ctx[:, ns * H:(ns + 1) * H],
                            op0=ALU.mult, op1=ALU.add,
                            accum_out=sum_b[:, nt_i:nt_i + 1])
                        v.tensor_scalar(mu_b[:, nt_i:nt_i + 1],
                                        sum_b[:, nt_i:nt_i + 1],
                                        1.0 / H, None, op0=ALU.mult)
                        sq = smpool.tile([128, H], F32, tag="sq")
                        v.scalar_tensor_tensor(
                            sq[:], cs, mu_b[:, nt_i:nt_i + 1], cs,
                            op0=ALU.subtract, op1=ALU.mult,
                            accum_out=var_b[:, nt_i:nt_i + 1])

                    # ---- per-chunk LayerNorm + relu + store ----
                    # last chunk: per-block stat chains (pipelines the tail)
                    parts = ([list(range(g * GS, (g + 1) * GS))] if g < NG - 1
                             else [[nt_i] for nt_i in
                                   range(g * GS, (g + 1) * GS)])
                    for part in parts:
                        emit_ln(part)
                    nc.sync.dma_start(out_v[:, g * GS:(g + 1) * GS, :],
                                      out_sv[:, g * GS:(g + 1) * GS, :])

                    if g == NG - 2:
                        # pre-emit chunk-3 direct masks for gg<=2: frees the
                        # e_g3 low slices for the next repeat's u-cells early
                        for gg in range(NG - 1):
                            attu = apool.tile([128, 4 * CHUNK], F16,
                                              tag="attu", name=f"attu3_{gg}e")
                            sl = gg * GS * CHUNK
                            ev = e_g[NG - 1][:, sl:sl + 4 * CHUNK].rearrange(
                                "p (b c) -> p b c", c=CHUNK)
                            av = at_tiles[NG - 1, gg][:].rearrange(
                                "p (b c) -> p b c", c=CHUNK)
                            ov = attu[:].rearrange("p (b c) -> p b c", c=CHUNK)
                            meng = g_ if (NG - 1, gg) in POOL_MASKS else v
                            meng.tensor_tensor(ov, ev, av, op=ALU.mult)
                            premask[NG - 1, gg] = attu

            for _rep in range(repeat):
                emit_all()

    nc.compile()
    nc.m = get_hw_module(nc.m)
    return nc


_cache = {}


def _get_program(apply_ln: bool):
    if apply_ln not in _cache:
        _cache[apply_ln] = build_program(apply_ln)
    return _cache[apply_ln]


def _prep_in_maps(hidden_state, adjacency_matrix, Wl, bl, Ws, bs, Wr, br,
                  ln_w, ln_b, apply_ln):
    f16 = np.float16
    tri = np.tri(128, 128, -1)
    wcat = np.concatenate([np.ascontiguousarray(Wl.T),
                           np.ascontiguousarray(Ws.T),
                           np.ascontiguousarray(Wr.T)], axis=1).astype(f16)
    bias_cat = np.concatenate([np.asarray(bl), np.asarray(bs),
                               np.asarray(br)]).astype(np.float32)
    shared = {
        "wcat": wcat,
        "bias_cat": np.broadcast_to(bias_cat, (128, 3 * H)).copy(),
        "masklo": tri.astype(np.uint8),   # [p,q]: 1 if p>q (m>n: left)
        "maskup": tri.T.astype(np.uint8), # 1 if p<q (m<n: right)
    }
    if apply_ln:
        shared["lnw_bc"] = np.broadcast_to(ln_w.astype(np.float32), (128, H)).copy()
        shared["lnb_bc"] = np.broadcast_to(ln_b.astype(np.float32), (128, H)).copy()
    import ml_dtypes
    f8 = ml_dtypes.float8_e4m3
    A16 = np.asarray(adjacency_matrix).astype(f16)
    h32 = np.asarray(hidden_state, dtype=np.float32)
    h16 = h32.astype(f16)
    in_maps = []
    for b in range(B):
        diag = np.diagonal(adjacency_matrix[b]).astype(np.float32)
        ud = (h16[b].astype(np.float32) ** 2).sum(axis=1)
        m = dict(shared)
        m["hT"] = np.ascontiguousarray(h16[b].T)
        hT32 = m["hT"].astype(np.float32)        # [256, 2048]
        h8 = hT32.astype(f8)
        r8 = (hT32 - h8.astype(np.float32)).astype(f8)
        m["h8"] = np.concatenate([h8[0:128], h8[128:256]], axis=1).copy()
        m["r8"] = np.concatenate([r8[0:128], r8[128:256]], axis=1).copy()
        m["AT"] = np.ascontiguousarray(A16[b].T)
        m["adiag"] = np.ascontiguousarray(diag.reshape(NT, 128).T)
        m["udiag"] = np.ascontiguousarray(ud.reshape(NT, 128).T.astype(np.float32))
        in_maps.append(m)
    return in_maps


def kernel(hidden_state, adjacency_matrix, Wl, bl, Ws, bs, Wr, br, ln_w, ln_b):
    apply_ln = not (np.all(np.asarray(ln_w) == 1.0)
                    and np.all(np.asarray(ln_b) == 0.0))
    nc = _get_program(apply_ln)
    in_maps = _prep_in_maps(hidden_state, adjacency_matrix, Wl, bl, Ws, bs,
                            Wr, br, ln_w, ln_b, apply_ln)
    res = bass_utils.run_bass_kernel_spmd(nc, in_maps, core_ids=list(range(B)))
    return np.stack([res.results[b]["out"] for b in range(B)]).astype(np.float32)

